# revision 20
# baseline (speedup 1.0000x reference)
"""CoDAConv2d Trainium2 kernel v2 (8-core SPMD, batch x H-halves).

Per-pixel reformulation (host-finish variant):
    V[(c',co)]  = sum_p w2[p,(c',co)] patches[p]          pass A (PE, K=116)
    mix rows    = [ norm2[co]-s | bias[co] ]              pass B (PE, M=32)
        norm2-s = quadratic form in x via host-shipped xx features
        bias    = sum_p bm[p,co] patches[p]
    prod        = V * xrep                                 (DVE)
    act partial = sum_c' prod[(c',co)]                     pass C (PE) --
        accumulated ONTO the bias rows of the same PSUM bank, so the
        device ships raw [norm2-s | act+bias] and the host finishes with
        out = (act+bias) / (sqrt(norm2) + 1e-6)  exactly (adds s, eps).

3 PE passes/chunk vs 5 in v1; no per-chunk ACT work; no on-device
normalize chain.  For the RSET chunks the xrep tile is built on the PE
from the q rows (R-pass, K=8) and staged to SBUF via an ACT copy (DVE
cannot read two PSUM operands), balancing PE (~9.0us) against
DMA_ENGINES (~8.8us).  R-pairs and D-pairs alternate in the stream so
the DVE mul supply mixes fast (DMA'd xrep) and slow (staged) chunks and
never starves.  All input DMAs ride the SP queue in consumption order
(HWDGE is a single global resource; an idle queue's transfer would jump
ahead on DMA_ENGINES).  Inputs arrive as 2-chunk pair DMAs into
pack-wide [128, 4, 896] tiles (slot = [q(448) | xrep(448)]).
"""

import numpy as np
from contextlib import ExitStack

C_IN = 8
C_OUT = 16
PATCH = 72
B = 4
H = W = 112
HALF = 56
NPX = HALF * W          # 6272
CH = 448
NCHUNK = NPX // CH      # 14
PACK = 4
NPACKS = 4              # packs 0-2 full, pack 3 = chunks 12,13
NXX = 36
NQ = C_IN + NXX + PATCH  # 116 rows of q
RSET = frozenset((2, 3, 6, 7))           # chunks whose xrep is PE-built
# (first-chunk-index, kind) per 2-chunk pair.  Chunks 0/1 ship with xrep as
# two single-chunk DMAs so the first DVE mul starts right after A0 (no
# R-pass/ACT-staging on the startup path); R-chunks (q-only, 319ns/chunk)
# follow so the stream stays ahead of PE; full D-pairs close it out.
PAIRS = [(0, 'DS'), (2, 'R'), (4, 'D'), (6, 'R'), (8, 'D'), (10, 'D'),
         (12, 'D')]
NCORES = 8

_CACHE = {}


def _build_program():
    if "nc" in _CACHE:
        return _CACHE["nc"]
    import concourse.bacc as bacc
    import concourse.tile as tile
    from concourse import mybir

    f32 = mybir.dt.float32
    bf16 = mybir.dt.bfloat16
    AF = mybir.ActivationFunctionType

    nc = bacc.Bacc("TRN2", target_bir_lowering=False, debug=False,
                   num_devices=NCORES)
    pk_d = nc.declare_dram_parameter("pk", [len(PAIRS) * 128, 1792], bf16,
                                     isOutput=False)
    wab_d = nc.declare_dram_parameter("wab", [128, 704], bf16, isOutput=False)
    out_d = nc.declare_dram_parameter("out", [4 * 128, CH], bf16,
                                      isOutput=True)

    with tile.TileContext(nc) as tc, ExitStack() as ctx:
        singles = ctx.enter_context(tc.tile_pool(name="singles", bufs=1))
        osb = ctx.enter_context(tc.tile_pool(name="osb", bufs=4))
        prodp = ctx.enter_context(tc.tile_pool(name="prodp", bufs=8))
        psv = ctx.enter_context(tc.tile_pool(name="psv", bufs=4, space="PSUM"))
        psm = ctx.enter_context(tc.tile_pool(name="psm", bufs=2, space="PSUM"))
        psxr = ctx.enter_context(tc.tile_pool(name="psxr", bufs=2,
                                              space="PSUM"))

        # --- input stream + static weights ---------------------------------
        pk_t = [singles.tile([128, PACK, 2 * CH], bf16, name=f"pk{p}")
                for p in range(NPACKS)]
        # SP front-loads one merged weight DMA then chunk 0/1 singles
        # (with xrep) so A0 ~4.0us and the first DVE mul ~4.2us.
        wab_sb = singles.tile([128, 704], bf16, name="wab")
        # c0+c1 as ONE D-pair: the 1274ns transfer exceeds the 650ns HWDGE
        # cadence, so p23 (which pins mul2, the DVE saturation head) becomes
        # transfer-bound at 3741 instead of slot-paced at 3916.
        nc.sync.dma_start(out=wab_sb[:], in_=wab_d[:])
        nc.sync.dma_start(out=pk_t[0][:, 0:2, :], in_=pk_d[0:128, 0:1792])

        junk = singles.tile([128, CH], bf16, name="junk")
        nc.gpsimd.memset(junk[:], 0)
        for w in range(5):
            wup = psv.tile([128, CH], f32, tag="v", name="wup")
            nc.tensor.matmul(wup[:], junk[0:128, 0:128], junk[:],
                             start=True, stop=True)

        # All input DMAs ride the SP queue in exact consumption order:
        # HWDGE serializes globally, so a single queue in order is the only
        # arrangement where no idle queue's transfer can jump ahead of a
        # startup-critical one on DMA_ENGINES.
        for j in range(1, len(PAIRS)):
            i0, kind = PAIRS[j]
            p, k = i0 // PACK, i0 % PACK
            if kind == 'D':
                nc.sync.dma_start(out=pk_t[p][:, k:k + 2, :],
                                  in_=pk_d[128 * j:128 * (j + 1), :])
            else:
                nc.sync.dma_start(out=pk_t[p][:, k:k + 2, 0:CH],
                                  in_=pk_d[128 * j:128 * (j + 1), 0:896])

        wA = wab_sb[0:NQ, 0:128]
        rst = wab_sb[0:C_IN, 128:256]
        # Bbuf region [256,480), content at [352,384): window 352-32k puts
        # the [norm|bias] block at output rows 32k..32k+32.
        bbuf = [wab_sb[0:NQ, 352 - 32 * k:480 - 32 * k] for k in range(PACK)]
        # Cbuf region [480,704), content at [592,608): window 576-32k puts
        # the act block at rows 32k+16 (accumulating onto the bias rows).
        cbuf = [wab_sb[0:128, 576 - 32 * k:704 - 32 * k] for k in range(PACK)]

        # --- main pipeline -------------------------------------------------
        mix = {}
        pend_c = []
        pend_out = [None]

        def flush(p):
            o = osb.tile([128, CH], bf16, tag="out", name=f"o{p}")
            rows = 128 if p < 3 else 64   # pack 3 uses blocks 0/1 only
            nc.scalar.activation(o[0:rows, :], mix[p][0:rows, :], AF.Copy,
                                 bias=0.0, scale=1.0)
            nc.sync.dma_start(out=out_d[128 * p:128 * p + rows, :],
                              in_=o[0:rows, :])

        staged = {}

        def stage_xrep(i):
            p, k = i // PACK, i % PACK
            xr = psxr.tile([128, CH], f32, tag="xr", name=f"xr{i}")
            nc.tensor.matmul(xr[:], rst, pk_t[p][0:C_IN, k, 0:CH],
                             start=True, stop=True)
            # DVE can't read two PSUM operands; stage xrep via ACT->SBUF
            xr_sb = prodp.tile([128, CH], bf16, tag="xrs", name=f"xs{i}")
            nc.scalar.activation(xr_sb[:], xr[:], AF.Copy, bias=0.0,
                                 scale=1.0)
            staged[i] = xr_sb

        def emit_chunk(i):
            p, k = i // PACK, i % PACK
            kp = PACK if p < 3 else 2
            q = pk_t[p][0:NQ, k, 0:CH]
            if i in RSET:
                if i not in staged:
                    stage_xrep(i)
                    if i + 1 in RSET:
                        stage_xrep(i + 1)
                xr_ap = staged.pop(i)[:]
            else:
                xr_ap = pk_t[p][:, k, CH:2 * CH]
            v = psv.tile([128, CH], f32, tag="v", name=f"v{i}")
            nc.tensor.matmul(v[:], wA, q, start=True, stop=True)
            if k == 0:
                mix[p] = psm.tile([128, CH], f32, tag="mix", name=f"mix{p}")
            nc.tensor.matmul(mix[p][:], bbuf[k], q,
                             start=(k == 0), stop=False)
            prod = prodp.tile([128, CH], bf16, tag="prod", name=f"pr{i}")
            nc.vector.tensor_mul(prod[:], v[:], xr_ap)
            pend_c.append((p, k, kp, prod))

        def emit_c():
            p, k, kp, prod = pend_c.pop(0)
            nc.tensor.matmul(mix[p][:], cbuf[k], prod[:],
                             start=False, stop=(k == kp - 1))
            if k == kp - 1:
                pend_out[0] = p

        for i in range(NCHUNK):
            emit_chunk(i)
            if i == 5:
                # hoist pair 6/7's R-passes + staging copies ahead of chunk
                # 6's emission: p67 lands ~7.4us but PE only reaches the
                # R6 pass at ~7.9 when emitted in-chunk; starting the
                # staging chain here pulls mul6 (the DVE-solid start that
                # paces the whole tail) in by ~0.6us.
                stage_xrep(6)
                stage_xrep(7)
            if i >= 2:
                emit_c()
            if pend_out[0] is not None:
                flush(pend_out[0])
                pend_out[0] = None
        while pend_c:
            emit_c()
            if pend_out[0] is not None and pend_c:
                flush(pend_out[0])
                pend_out[0] = None
        flush(pend_out[0])

    nc.compile()
    _CACHE["nc"] = nc
    return nc


def make_weights(w_pred, b_pred):
    """-> (wa [128,256] = [wA | R], wb2 [128,448] = [Bbuf | Cbuf]) bf16
    plus host constant s[co]."""
    import ml_dtypes
    w_pred = np.asarray(w_pred, dtype=np.float64)
    b_pred = np.asarray(b_pred, dtype=np.float64)
    wr = w_pred.reshape(PATCH, C_OUT, C_IN)        # [p, co, c]
    bm = b_pred.reshape(PATCH, C_OUT)              # [p, co]
    w2 = np.ascontiguousarray(wr.transpose(0, 2, 1)).reshape(
        PATCH, C_IN * C_OUT)                       # [p, (c',co)]
    A = np.einsum('poc,pod->ocd', wr, wr)          # [co, 8, 8]
    u = np.einsum('po,poc->oc', bm, wr)            # [co, 8]
    s = np.einsum('po,po->o', bm, bm)              # [co]

    wab = np.zeros((128, 704), dtype=np.float64)
    wab[C_IN + NXX:NQ, 0:128] = w2                 # wA (patch rows)
    for cp in range(C_IN):                         # R: x row c' -> (c',co)
        for co in range(C_OUT):
            wab[cp, 128 + cp * C_OUT + co] = 1.0
    bc = 352                                       # Bbuf content col
    pairs = [(c, d) for c in range(C_IN) for d in range(c, C_IN)]
    for co in range(C_OUT):
        for c in range(C_IN):
            wab[c, bc + co] = 2.0 * u[co, c]
        for j, (c, d) in enumerate(pairs):
            wab[C_IN + j, bc + co] = A[co, c, d] * (2.0 if c != d else 1.0)
        for p in range(PATCH):
            wab[C_IN + NXX + p, bc + 16 + co] = bm[p, co]
    cc = 592                                       # Cbuf content col
    for cp in range(C_IN):
        for co in range(C_OUT):
            wab[cp * C_OUT + co, cc + co] = 1.0
    return np.ascontiguousarray(wab, dtype=ml_dtypes.bfloat16), s


def make_shard_inputs(in_tensor, core):
    """-> pk bf16 [len(PAIRS)*128, 1792], pair rows = [chunk i | chunk i+1],
    each chunk block = [q(448) | xrep(448)] (R-pairs: [q_i | q_i+1])."""
    import ml_dtypes
    b, sgn = core // 2, core % 2
    r0 = sgn * HALF
    pad = np.zeros((C_IN, H + 2, W + 2), dtype=np.float32)
    pad[:, 1:1 + H, 1:1 + W] = in_tensor[b]
    pat = np.empty((C_IN, 3, 3, HALF, W), dtype=np.float32)
    for di in range(3):
        for dj in range(3):
            pat[:, di, dj] = pad[:, r0 + di:r0 + di + HALF, dj:dj + W]
    pat = pat.reshape(PATCH, NPX)
    xin = in_tensor[b, :, r0:r0 + HALF, :].reshape(C_IN, NPX)
    pairs = [(c, d) for c in range(C_IN) for d in range(c, C_IN)]
    xx = np.empty((NXX, NPX), dtype=np.float32)
    for j, (c, d) in enumerate(pairs):
        xx[j] = xin[c] * xin[d]
    xr16 = np.repeat(xin, C_OUT, axis=0)           # [(c',co), n]

    def qblk(i):
        c0, c1 = i * CH, (i + 1) * CH
        q = np.zeros((128, CH), dtype=np.float32)
        q[0:C_IN] = xin[:, c0:c1]
        q[C_IN:C_IN + NXX] = xx[:, c0:c1]
        q[C_IN + NXX:NQ] = pat[:, c0:c1]
        return q

    pk = np.zeros((len(PAIRS), 128, 1792), dtype=np.float32)
    for j, (i0, kind) in enumerate(PAIRS):
        if kind in ('D', 'DS'):
            pk[j, :, 0:CH] = qblk(i0)
            pk[j, :, CH:2 * CH] = xr16[:, i0 * CH:(i0 + 1) * CH]
            pk[j, :, 2 * CH:3 * CH] = qblk(i0 + 1)
            pk[j, :, 3 * CH:4 * CH] = xr16[:, (i0 + 1) * CH:(i0 + 2) * CH]
        else:
            pk[j, :, 0:CH] = qblk(i0)
            pk[j, :, CH:2 * CH] = qblk(i0 + 1)
    return np.ascontiguousarray(
        pk.reshape(len(PAIRS) * 128, 1792)).astype(ml_dtypes.bfloat16)


def unscramble(res_bf, s):
    """[4*128,CH] bf16 -> [C_OUT, HALF, W] f32 host-finish."""
    v = np.asarray(res_bf, dtype=np.float32).reshape(4, 4, 32, CH)
    res = np.empty((C_OUT, NCHUNK, CH), dtype=np.float32)
    for i in range(NCHUNK):
        p, k = i // PACK, i % PACK
        blk = v[p, k]
        n2 = blk[0:16, :] + s[:, None]
        ab = blk[16:32, :]
        res[:, i, :] = ab / (np.sqrt(np.maximum(n2, 0.0)) + 1e-6)
    return res.reshape(C_OUT, NPX).reshape(C_OUT, HALF, W)


def kernel(in_tensor, w_pred, b_pred):
    from concourse.bass_utils import run_bass_kernel_spmd

    in_tensor = np.asarray(in_tensor, dtype=np.float32)
    nc = _build_program()
    wab, s = make_weights(w_pred, b_pred)
    in_maps = [{"pk": make_shard_inputs(in_tensor, c), "wab": wab}
               for c in range(NCORES)]
    res = run_bass_kernel_spmd(nc, in_maps, list(range(NCORES)))
    out = np.empty((B, C_OUT, H, W), dtype=np.float32)
    for c in range(NCORES):
        b, sgn = c // 2, c % 2
        out[b, :, sgn * HALF:(sgn + 1) * HALF, :] = \
            unscramble(res.results[c]["out"], s)
    return out


# revision 21
# speedup vs baseline: 1.0081x; 1.0081x over previous
"""CoDAConv2d Trainium2 kernel v2 (8-core SPMD, batch x H-halves).

Per-pixel reformulation (host-finish variant):
    V[(c',co)]  = sum_p w2[p,(c',co)] patches[p]          pass A (PE, K=116)
    mix rows    = [ norm2[co]-s | bias[co] ]              pass B (PE, M=32)
        norm2-s = quadratic form in x via host-shipped xx features
        bias    = sum_p bm[p,co] patches[p]
    prod        = V * xrep                                 (DVE)
    act partial = sum_c' prod[(c',co)]                     pass C (PE) --
        accumulated ONTO the bias rows of the same PSUM bank, so the
        device ships raw [norm2-s | act+bias] and the host finishes with
        out = (act+bias) / (sqrt(norm2) + 1e-6)  exactly (adds s, eps).

3 PE passes/chunk vs 5 in v1; no per-chunk ACT work; no on-device
normalize chain.  For the RSET chunks the xrep tile is built on the PE
from the q rows (R-pass, K=8) and staged to SBUF via an ACT copy (DVE
cannot read two PSUM operands), balancing PE (~9.0us) against
DMA_ENGINES (~8.8us).  R-pairs and D-pairs alternate in the stream so
the DVE mul supply mixes fast (DMA'd xrep) and slow (staged) chunks and
never starves.  All input DMAs ride the SP queue in consumption order
(HWDGE is a single global resource; an idle queue's transfer would jump
ahead on DMA_ENGINES).  Inputs arrive as 2-chunk pair DMAs into
pack-wide [128, 4, 896] tiles (slot = [q(448) | xrep(448)]).
"""

import numpy as np
from contextlib import ExitStack

C_IN = 8
C_OUT = 16
PATCH = 72
B = 4
H = W = 112
HALF = 56
NPX = HALF * W          # 6272
CH = 448
NCHUNK = NPX // CH      # 14
PACK = 4
NPACKS = 4              # packs 0-2 full, pack 3 = chunks 12,13
NXX = 36
NQ = C_IN + NXX + PATCH  # 116 rows of q
RSET = frozenset((2, 3, 6, 7))           # chunks whose xrep is PE-built
# (first-chunk-index, kind) per 2-chunk pair.  Chunks 0/1 ship with xrep as
# two single-chunk DMAs so the first DVE mul starts right after A0 (no
# R-pass/ACT-staging on the startup path); R-chunks (q-only, 319ns/chunk)
# follow so the stream stays ahead of PE; full D-pairs close it out.
PAIRS = [(0, 'DS'), (2, 'R'), (4, 'D'), (6, 'R'), (8, 'D'), (10, 'D'),
         (12, 'D')]
NCORES = 8

_CACHE = {}


def _build_program():
    if "nc" in _CACHE:
        return _CACHE["nc"]
    import concourse.bacc as bacc
    import concourse.tile as tile
    from concourse import mybir

    f32 = mybir.dt.float32
    bf16 = mybir.dt.bfloat16
    AF = mybir.ActivationFunctionType

    nc = bacc.Bacc("TRN2", target_bir_lowering=False, debug=False,
                   num_devices=NCORES)
    pk_d = nc.declare_dram_parameter("pk", [len(PAIRS) * 128, 1792], bf16,
                                     isOutput=False)
    wab_d = nc.declare_dram_parameter("wab", [128, 704], bf16, isOutput=False)
    out_d = nc.declare_dram_parameter("out", [4 * 128, CH], bf16,
                                      isOutput=True)

    with tile.TileContext(nc) as tc, ExitStack() as ctx:
        singles = ctx.enter_context(tc.tile_pool(name="singles", bufs=1))
        osb = ctx.enter_context(tc.tile_pool(name="osb", bufs=4))
        prodp = ctx.enter_context(tc.tile_pool(name="prodp", bufs=8))
        psv = ctx.enter_context(tc.tile_pool(name="psv", bufs=4, space="PSUM"))
        psm = ctx.enter_context(tc.tile_pool(name="psm", bufs=2, space="PSUM"))
        psxr = ctx.enter_context(tc.tile_pool(name="psxr", bufs=2,
                                              space="PSUM"))

        # --- input stream + static weights ---------------------------------
        pk_t = [singles.tile([128, PACK, 2 * CH], bf16, name=f"pk{p}")
                for p in range(NPACKS)]
        # SP front-loads one merged weight DMA then chunk 0/1 singles
        # (with xrep) so A0 ~4.0us and the first DVE mul ~4.2us.
        wab_sb = singles.tile([128, 704], bf16, name="wab")
        # c0 (637ns >= the 625ns HWDGE cadence) goes FIRST: a sub-cadence
        # first transfer (wab, 501ns) leaves a pacing bubble that every
        # downstream arrival inherits, including p23 which pins mul2 (the
        # head of the DVE saturation chain).
        nc.sync.dma_start(out=pk_t[0][:, 0, :], in_=pk_d[0:128, 0:896])
        nc.sync.dma_start(out=wab_sb[:], in_=wab_d[:])
        nc.sync.dma_start(out=pk_t[0][:, 1, :], in_=pk_d[0:128, 896:1792])

        junk = singles.tile([128, CH], bf16, name="junk")
        nc.gpsimd.memset(junk[:], 0)
        for w in range(5):
            wup = psv.tile([128, CH], f32, tag="v", name="wup")
            nc.tensor.matmul(wup[:], junk[0:128, 0:128], junk[:],
                             start=True, stop=True)

        # All input DMAs ride the SP queue in exact consumption order:
        # HWDGE serializes globally, so a single queue in order is the only
        # arrangement where no idle queue's transfer can jump ahead of a
        # startup-critical one on DMA_ENGINES.
        for j in range(1, len(PAIRS)):
            i0, kind = PAIRS[j]
            p, k = i0 // PACK, i0 % PACK
            if kind == 'D':
                nc.sync.dma_start(out=pk_t[p][:, k:k + 2, :],
                                  in_=pk_d[128 * j:128 * (j + 1), :])
            else:
                nc.sync.dma_start(out=pk_t[p][:, k:k + 2, 0:CH],
                                  in_=pk_d[128 * j:128 * (j + 1), 0:896])

        wA = wab_sb[0:NQ, 0:128]
        rst = wab_sb[0:C_IN, 128:256]
        # Bbuf region [256,480), content at [352,384): window 352-32k puts
        # the [norm|bias] block at output rows 32k..32k+32.
        bbuf = [wab_sb[0:NQ, 352 - 32 * k:480 - 32 * k] for k in range(PACK)]
        # Cbuf region [480,704), content at [592,608): window 576-32k puts
        # the act block at rows 32k+16 (accumulating onto the bias rows).
        cbuf = [wab_sb[0:128, 576 - 32 * k:704 - 32 * k] for k in range(PACK)]

        # --- main pipeline -------------------------------------------------
        mix = {}
        pend_c = []
        pend_out = [None]

        def flush(p):
            o = osb.tile([128, CH], bf16, tag="out", name=f"o{p}")
            rows = 128 if p < 3 else 64   # pack 3 uses blocks 0/1 only
            nc.scalar.activation(o[0:rows, :], mix[p][0:rows, :], AF.Copy,
                                 bias=0.0, scale=1.0)
            nc.sync.dma_start(out=out_d[128 * p:128 * p + rows, :],
                              in_=o[0:rows, :])

        staged = {}

        def stage_xrep(i):
            p, k = i // PACK, i % PACK
            xr = psxr.tile([128, CH], f32, tag="xr", name=f"xr{i}")
            nc.tensor.matmul(xr[:], rst, pk_t[p][0:C_IN, k, 0:CH],
                             start=True, stop=True)
            # DVE can't read two PSUM operands; stage xrep via ACT->SBUF
            xr_sb = prodp.tile([128, CH], bf16, tag="xrs", name=f"xs{i}")
            nc.scalar.activation(xr_sb[:], xr[:], AF.Copy, bias=0.0,
                                 scale=1.0)
            staged[i] = xr_sb

        def emit_chunk(i):
            p, k = i // PACK, i % PACK
            kp = PACK if p < 3 else 2
            q = pk_t[p][0:NQ, k, 0:CH]
            if i in RSET:
                if i not in staged:
                    stage_xrep(i)
                    if i + 1 in RSET:
                        stage_xrep(i + 1)
                xr_ap = staged.pop(i)[:]
            else:
                xr_ap = pk_t[p][:, k, CH:2 * CH]
            v = psv.tile([128, CH], f32, tag="v", name=f"v{i}")
            nc.tensor.matmul(v[:], wA, q, start=True, stop=True)
            if k == 0:
                mix[p] = psm.tile([128, CH], f32, tag="mix", name=f"mix{p}")
            nc.tensor.matmul(mix[p][:], bbuf[k], q,
                             start=(k == 0), stop=False)
            prod = prodp.tile([128, CH], bf16, tag="prod", name=f"pr{i}")
            nc.vector.tensor_mul(prod[:], v[:], xr_ap)
            pend_c.append((p, k, kp, prod))

        def emit_c():
            p, k, kp, prod = pend_c.pop(0)
            nc.tensor.matmul(mix[p][:], cbuf[k], prod[:],
                             start=False, stop=(k == kp - 1))
            if k == kp - 1:
                pend_out[0] = p

        for i in range(NCHUNK):
            emit_chunk(i)
            if i == 5:
                # hoist pair 6/7's R-passes + staging copies ahead of chunk
                # 6's emission: p67 lands ~7.4us but PE only reaches the
                # R6 pass at ~7.9 when emitted in-chunk; starting the
                # staging chain here pulls mul6 (the DVE-solid start that
                # paces the whole tail) in by ~0.6us.
                stage_xrep(6)
                stage_xrep(7)
            if i >= 2:
                emit_c()
            if pend_out[0] is not None:
                flush(pend_out[0])
                pend_out[0] = None
        while pend_c:
            emit_c()
            if pend_out[0] is not None and pend_c:
                flush(pend_out[0])
                pend_out[0] = None
        flush(pend_out[0])

    nc.compile()
    _CACHE["nc"] = nc
    return nc


def make_weights(w_pred, b_pred):
    """-> (wa [128,256] = [wA | R], wb2 [128,448] = [Bbuf | Cbuf]) bf16
    plus host constant s[co]."""
    import ml_dtypes
    w_pred = np.asarray(w_pred, dtype=np.float64)
    b_pred = np.asarray(b_pred, dtype=np.float64)
    wr = w_pred.reshape(PATCH, C_OUT, C_IN)        # [p, co, c]
    bm = b_pred.reshape(PATCH, C_OUT)              # [p, co]
    w2 = np.ascontiguousarray(wr.transpose(0, 2, 1)).reshape(
        PATCH, C_IN * C_OUT)                       # [p, (c',co)]
    A = np.einsum('poc,pod->ocd', wr, wr)          # [co, 8, 8]
    u = np.einsum('po,poc->oc', bm, wr)            # [co, 8]
    s = np.einsum('po,po->o', bm, bm)              # [co]

    wab = np.zeros((128, 704), dtype=np.float64)
    wab[C_IN + NXX:NQ, 0:128] = w2                 # wA (patch rows)
    for cp in range(C_IN):                         # R: x row c' -> (c',co)
        for co in range(C_OUT):
            wab[cp, 128 + cp * C_OUT + co] = 1.0
    bc = 352                                       # Bbuf content col
    pairs = [(c, d) for c in range(C_IN) for d in range(c, C_IN)]
    for co in range(C_OUT):
        for c in range(C_IN):
            wab[c, bc + co] = 2.0 * u[co, c]
        for j, (c, d) in enumerate(pairs):
            wab[C_IN + j, bc + co] = A[co, c, d] * (2.0 if c != d else 1.0)
        for p in range(PATCH):
            wab[C_IN + NXX + p, bc + 16 + co] = bm[p, co]
    cc = 592                                       # Cbuf content col
    for cp in range(C_IN):
        for co in range(C_OUT):
            wab[cp * C_OUT + co, cc + co] = 1.0
    return np.ascontiguousarray(wab, dtype=ml_dtypes.bfloat16), s


def make_shard_inputs(in_tensor, core):
    """-> pk bf16 [len(PAIRS)*128, 1792], pair rows = [chunk i | chunk i+1],
    each chunk block = [q(448) | xrep(448)] (R-pairs: [q_i | q_i+1])."""
    import ml_dtypes
    b, sgn = core // 2, core % 2
    r0 = sgn * HALF
    pad = np.zeros((C_IN, H + 2, W + 2), dtype=np.float32)
    pad[:, 1:1 + H, 1:1 + W] = in_tensor[b]
    pat = np.empty((C_IN, 3, 3, HALF, W), dtype=np.float32)
    for di in range(3):
        for dj in range(3):
            pat[:, di, dj] = pad[:, r0 + di:r0 + di + HALF, dj:dj + W]
    pat = pat.reshape(PATCH, NPX)
    xin = in_tensor[b, :, r0:r0 + HALF, :].reshape(C_IN, NPX)
    pairs = [(c, d) for c in range(C_IN) for d in range(c, C_IN)]
    xx = np.empty((NXX, NPX), dtype=np.float32)
    for j, (c, d) in enumerate(pairs):
        xx[j] = xin[c] * xin[d]
    xr16 = np.repeat(xin, C_OUT, axis=0)           # [(c',co), n]

    def qblk(i):
        c0, c1 = i * CH, (i + 1) * CH
        q = np.zeros((128, CH), dtype=np.float32)
        q[0:C_IN] = xin[:, c0:c1]
        q[C_IN:C_IN + NXX] = xx[:, c0:c1]
        q[C_IN + NXX:NQ] = pat[:, c0:c1]
        return q

    pk = np.zeros((len(PAIRS), 128, 1792), dtype=np.float32)
    for j, (i0, kind) in enumerate(PAIRS):
        if kind in ('D', 'DS'):
            pk[j, :, 0:CH] = qblk(i0)
            pk[j, :, CH:2 * CH] = xr16[:, i0 * CH:(i0 + 1) * CH]
            pk[j, :, 2 * CH:3 * CH] = qblk(i0 + 1)
            pk[j, :, 3 * CH:4 * CH] = xr16[:, (i0 + 1) * CH:(i0 + 2) * CH]
        else:
            pk[j, :, 0:CH] = qblk(i0)
            pk[j, :, CH:2 * CH] = qblk(i0 + 1)
    return np.ascontiguousarray(
        pk.reshape(len(PAIRS) * 128, 1792)).astype(ml_dtypes.bfloat16)


def unscramble(res_bf, s):
    """[4*128,CH] bf16 -> [C_OUT, HALF, W] f32 host-finish."""
    v = np.asarray(res_bf, dtype=np.float32).reshape(4, 4, 32, CH)
    res = np.empty((C_OUT, NCHUNK, CH), dtype=np.float32)
    for i in range(NCHUNK):
        p, k = i // PACK, i % PACK
        blk = v[p, k]
        n2 = blk[0:16, :] + s[:, None]
        ab = blk[16:32, :]
        res[:, i, :] = ab / (np.sqrt(np.maximum(n2, 0.0)) + 1e-6)
    return res.reshape(C_OUT, NPX).reshape(C_OUT, HALF, W)


def kernel(in_tensor, w_pred, b_pred):
    from concourse.bass_utils import run_bass_kernel_spmd

    in_tensor = np.asarray(in_tensor, dtype=np.float32)
    nc = _build_program()
    wab, s = make_weights(w_pred, b_pred)
    in_maps = [{"pk": make_shard_inputs(in_tensor, c), "wab": wab}
               for c in range(NCORES)]
    res = run_bass_kernel_spmd(nc, in_maps, list(range(NCORES)))
    out = np.empty((B, C_OUT, H, W), dtype=np.float32)
    for c in range(NCORES):
        b, sgn = c // 2, c % 2
        out[b, :, sgn * HALF:(sgn + 1) * HALF, :] = \
            unscramble(res.results[c]["out"], s)
    return out
